# revision 1
# baseline (speedup 1.0000x reference)
"""Trainium2 Bass kernel for nn_EnhancedSmagorinsky (B=1024, N=16384, f32).

Strategy (8 cores, pure batch-parallel, 128 batch rows per core):
- All convs become band-matrix matmuls in a position-major layout.
- Per tile of 107 output positions: PE-transpose a 127-position x window
  (plus a constant ones row used to inject biases through the matmuls),
  then chain: fused conv1 (deriv/box composed into one 9-tap kernel),
  conv2, conv3, the xdiff deriv, and the final deriv conv (computed as
  y_tile^T @ band, which lands row-major for direct store).
- ELU(z) computed exactly as max(z+1, min(exp(z), 1)) - 1 with the +1
  shift absorbed into the next layer's bias: 1 ScalarE exp + 1 fused
  DVE scalar_tensor_tensor per element.
- clip(elu(z),0,1) == clip(z,0,1) exactly, so layer 3 needs no exp.
"""
import json
import math

import numpy as np

# ---- problem constants (hardcoded; kernel.py must be self-contained) ----
B = 1024
N = 16384
NCORES = 8
BL = B // NCORES          # 128 batch rows per core
SCALE = 1.0
DX = SCALE / N
CUTOFF = 2.0 * DX
SQRT2 = float(np.sqrt(2.0))

# tile geometry
W = 127                   # x-window rows per tile (row 127 = ones)
S = 107                   # output positions per tile
PAD_L = 10
NT = math.ceil(N / S)     # 154
NEED_R = S * (NT - 1) + W - N
NPAD = PAD_L + N + NEED_R
GT = 4                    # tiles per group (free dim = GT*128 = 512)
NGROUPS = math.ceil(NT / GT)

# band-matrix column offsets inside the packed "bands" tensor
O_B1 = [co * 119 for co in range(6)]
O_XD = 6 * 119
O_B2 = [[O_XD + 111 + (ci * 3 + co) * 115 for co in range(3)] for ci in range(6)]
O_B3 = [O_XD + 111 + 18 * 115 + ci * 111 for ci in range(3)]
O_B5 = O_B3[0] + 3 * 111
O_ID = O_B5 + 108
O_ONES = O_ID + 128
NB = O_ONES + 512


def _build_constants(deriv_w, filt_w, W1, b1, W2, b2, W3, b3):
    """Pack all band matrices into one [128, NB] f32 tensor."""
    dw = np.asarray(deriv_w).reshape(5).astype(np.float64)
    bw = np.asarray(filt_w).reshape(3).astype(np.float64)
    W1 = np.asarray(W1).astype(np.float64)
    W2 = np.asarray(W2).astype(np.float64)
    W3 = np.asarray(W3).astype(np.float64)
    b1 = np.asarray(b1).astype(np.float64)
    b2 = np.asarray(b2).astype(np.float64)
    b3 = np.asarray(b3).astype(np.float64)

    F = -bw.copy()
    F[1] += 1.0
    K1 = np.zeros((6, 9))
    for co in range(6):
        K1[co] += np.convolve(W1[co, 0], dw)
        K1[co, 1:8] += np.convolve(W1[co, 1], F)
    shift1 = b1 + 1.0
    shift2 = b2 + 1.0 - W2.sum(axis=(1, 2))
    shift3 = b3[0] + 1.0 - W3.sum()

    bands = np.zeros((128, NB))
    for co in range(6):
        Bm = bands[:, O_B1[co]: O_B1[co] + 119]
        for r1 in range(119):
            for m in range(9):
                Bm[r1 + m, r1] = K1[co, m]
        Bm[127, :] = shift1[co]
    Bm = bands[:, O_XD: O_XD + 111]
    for r in range(111):
        for m in range(5):
            Bm[r + 6 + m, r] = dw[m]
    for ci in range(6):
        for co in range(3):
            Bm = bands[:, O_B2[ci][co]: O_B2[ci][co] + 115]
            for r2 in range(115):
                for m in range(5):
                    Bm[r2 + m, r2] = W2[co, ci, m]
            if ci == 5:
                Bm[119, :] = shift2[co]
    for ci in range(3):
        Bm = bands[:, O_B3[ci]: O_B3[ci] + 111]
        for r3 in range(111):
            for m in range(5):
                Bm[r3 + m, r3] = W3[0, ci, m]
        if ci == 2:
            Bm[115, :] = shift3
    Bm = bands[:, O_B5: O_B5 + 107]
    for np_ in range(107):
        for m in range(5):
            Bm[np_ + m, np_] = dw[m] * (CUTOFF ** 2) * SQRT2
    bands[:, O_ID: O_ID + 128] = np.eye(128)
    bands[0, O_ONES: O_ONES + 512] = 1.0
    return bands.astype(np.float32)


# ---- BIR fix: this walrus build allows only one embedded sync-wait per
# instruction; hoist extras onto standalone EventSemaphore instructions ----
def _split_multiwait_bir(bir_bytes: bytes) -> bytes:
    bir = json.loads(bir_bytes)
    ctr = 0
    for fn in bir.get("functions", []):
        for blk in fn.get("blocks", []):
            out = []
            for inst in blk.get("instructions", []):
                si = inst.get("sync_info")
                if si:
                    waits = si.get("on_wait") or []
                    if len(waits) > 1:
                        for w in waits[:-1]:
                            ctr += 1
                            out.append({
                                "debug": inst.get("debug", 0),
                                "engine": inst["engine"],
                                "ins": [], "outs": [],
                                "name": f"xwait-{ctr}",
                                "opcode": "EventSemaphore",
                                "sync_info": {"on_update": [], "on_wait": [w]},
                            })
                        si["on_wait"] = [waits[-1]]
                out.append(inst)
            blk["instructions"] = out
    return json.dumps(bir).encode()


_CACHED_NC = None


def _build_bass():
    global _CACHED_NC
    if _CACHED_NC is not None:
        return _CACHED_NC
    from contextlib import ExitStack

    import concourse.bass as bass
    import concourse.tile as tile
    from concourse import mybir
    from concourse.alu_op_type import AluOpType

    F32 = mybir.dt.float32
    F32R = mybir.dt.float32  # f32r loses ~13 mantissa bits -> 1.3e-2 rel err; keep f32
    AF = mybir.ActivationFunctionType

    nc = bass.Bass()
    xpD = nc.dram_tensor("xp", [BL, NPAD], F32R, kind="ExternalInput")
    bandsD = nc.dram_tensor("bands", [128, NB], F32R, kind="ExternalInput")
    outD = nc.dram_tensor("out", [BL, N], F32, kind="ExternalOutput")

    with ExitStack() as ctx:
        tc = ctx.enter_context(tile.TileContext(nc))
        const = ctx.enter_context(tc.tile_pool(name="const", bufs=1))
        sb = ctx.enter_context(tc.tile_pool(name="sb", bufs=2))
        ps = ctx.enter_context(tc.tile_pool(name="ps", bufs=1, space="PSUM"))

        x_sb = const.tile([BL, NPAD], F32R, tag="x_sb")
        nch = 8
        csz = (NPAD + nch - 1) // nch
        for ci_ in range(nch):
            c0 = ci_ * csz
            c1 = min(NPAD, c0 + csz)
            nc.sync.dma_start(out=x_sb[:, c0:c1], in_=xpD[:, c0:c1])
        bands = const.tile([128, NB], F32R, tag="bands")
        nc.sync.dma_start(out=bands, in_=bandsD[:])
        neg1 = const.tile([128, 1], F32, tag="neg1")
        nc.vector.memset(neg1, -1.0)
        identr = bands[:128, O_ID: O_ID + 128]
        onesr = bands[0:1, O_ONES: O_ONES + 512]

        Bxd = bands[:128, O_XD: O_XD + 111]
        B5 = bands[:111, O_B5: O_B5 + 107]

        # prewarm: constant ones rows live at fixed pool-slot addresses
        for _ in range(2):
            t_ = sb.tile([128, 512], F32R, tag="xpos")
            nc.sync.dma_start(out=t_[127:128, :], in_=onesr[:, :])
            t_ = sb.tile([120, 512], F32R, tag="h1_5")
            nc.sync.dma_start(out=t_[119:120, :], in_=onesr[:, :])
            t_ = sb.tile([116, 512], F32R, tag="h2_2")
            nc.sync.dma_start(out=t_[115:116, :], in_=onesr[:, :])

        def group(g):
            t0 = g * GT
            gt = min(GT, NT - t0)
            Fc = gt * 128
            # --- transpose x windows into position-major ---
            tp = ps.tile([127, 512], F32R, tag="tp")
            for j in range(gt):
                t = t0 + j
                nc.tensor.transpose(
                    tp[:, 128 * j: 128 * (j + 1)],
                    x_sb[:, S * t: S * t + W],
                    identr,
                )
            xt = sb.tile([128, 512], F32R, tag="xpos")
            nc.vector.tensor_copy(xt[:127, :Fc], tp[:127, :Fc])
            # --- xdiff ---
            xd = ps.tile([111, 512], F32, tag="xd")
            nc.tensor.matmul(xd[:, :Fc], lhsT=Bxd, rhs=xt[:, :Fc],
                             start=True, stop=True)
            # --- conv1 (fused 9-tap) + ELU ---
            h1 = []
            for co in range(6):
                z = ps.tile([119, 512], F32, tag="z1", bufs=2)
                nc.tensor.matmul(
                    z[:, :Fc], lhsT=bands[:128, O_B1[co]: O_B1[co] + 119],
                    rhs=xt[:, :Fc], start=True, stop=True)
                e = sb.tile([119, 512], F32, tag="e1", bufs=3)
                nc.scalar.activation(e[:, :Fc], z[:, :Fc], AF.Exp,
                                     bias=neg1[:119], scale=1.0)
                rows = 120 if co == 5 else 119
                h = sb.tile([rows, 512], F32R, tag=f"h1_{co}")
                nc.vector.scalar_tensor_tensor(
                    out=h[:119, :Fc], in0=e[:, :Fc], scalar=1.0,
                    in1=z[:, :Fc], op0=AluOpType.min, op1=AluOpType.max)
                h1.append(h)
            # --- conv2 + ELU ---
            h2 = []
            for co in range(3):
                z = ps.tile([115, 512], F32, tag="z2", bufs=2)
                for ci in range(6):
                    K = 120 if ci == 5 else 119
                    nc.tensor.matmul(
                        z[:, :Fc],
                        lhsT=bands[:K, O_B2[ci][co]: O_B2[ci][co] + 115],
                        rhs=h1[ci][:K, :Fc], start=(ci == 0), stop=(ci == 5))
                e = sb.tile([115, 512], F32, tag="e2", bufs=3)
                nc.scalar.activation(e[:, :Fc], z[:, :Fc], AF.Exp,
                                     bias=neg1[:115], scale=1.0)
                rows = 116 if co == 2 else 115
                h = sb.tile([rows, 512], F32R, tag=f"h2_{co}")
                nc.vector.scalar_tensor_tensor(
                    out=h[:115, :Fc], in0=e[:, :Fc], scalar=1.0,
                    in1=z[:, :Fc], op0=AluOpType.min, op1=AluOpType.max)
                h2.append(h)
            # --- conv3; clip(elu(z),0,1) == clip(z,0,1), so no exp ---
            z3 = ps.tile([111, 512], F32, tag="z3")
            for ci in range(3):
                K = 116 if ci == 2 else 115
                nc.tensor.matmul(
                    z3[:, :Fc], lhsT=bands[:K, O_B3[ci]: O_B3[ci] + 111],
                    rhs=h2[ci][:K, :Fc], start=(ci == 0), stop=(ci == 2))
            u = sb.tile([111, 512], F32, tag="u")       # cs + 1
            nc.vector.tensor_scalar(out=u[:, :Fc], in0=z3[:, :Fc],
                                    scalar1=1.0, scalar2=2.0,
                                    op0=AluOpType.max, op1=AluOpType.min)
            sq = sb.tile([111, 512], F32, tag="sq")     # cs^2
            nc.scalar.activation(sq[:, :Fc], u[:, :Fc], AF.Square,
                                 bias=neg1[:111], scale=1.0)
            ab = sb.tile([111, 512], F32, tag="ab")     # |xd|
            nc.scalar.activation(ab[:, :Fc], xd[:, :Fc], AF.Abs)
            v = sb.tile([111, 512], F32, tag="v")       # |xd|*xd
            nc.vector.tensor_tensor(v[:, :Fc], ab[:, :Fc], xd[:, :Fc],
                                    AluOpType.mult)
            y = sb.tile([111, 512], F32R, tag="y")
            nc.vector.tensor_tensor(y[:, :Fc], v[:, :Fc], sq[:, :Fc],
                                    AluOpType.mult)
            # --- final deriv conv, row-major out via y^T @ B5 ---
            ops = ps.tile([128, 512], F32, tag="ops")
            njs = []
            for j in range(gt):
                nj = min(S, N - S * (t0 + j))
                if nj <= 0:
                    continue
                n_mm = nj + (nj & 1)
                nc.tensor.matmul(
                    ops[:, 108 * j: 108 * j + n_mm],
                    lhsT=y[:111, 128 * j: 128 * j + 128],
                    rhs=bands[:111, O_B5: O_B5 + n_mm], start=True, stop=True)
                njs.append(nj)
            cols = 108 * (len(njs) - 1) + njs[-1]
            osb = sb.tile([128, 440], F32, tag="osb")
            nc.vector.tensor_copy(osb[:, :cols], ops[:, :cols])
            if all(nj == S for nj in njs):
                src_ap = osb[:, :gt * 108].rearrange(
                    "p (g s) -> p g s", s=108)[:, :, :S]
                dst_ap = outD[:, S * t0: S * (t0 + gt)].rearrange(
                    "p (g s) -> p g s", s=S)
                nc.sync.dma_start(out=dst_ap, in_=src_ap)
            else:
                for j, nj in enumerate(njs):
                    nc.sync.dma_start(
                        out=outD[:, S * (t0 + j): S * (t0 + j) + nj],
                        in_=osb[:, 108 * j: 108 * j + nj])

        for g in range(NGROUPS):
            group(g)

    orig = nc.to_json_bytes
    nc.to_json_bytes = lambda: _split_multiwait_bir(orig())
    _CACHED_NC = nc
    return nc


def kernel(**inputs) -> np.ndarray:
    from concourse.bass_utils import run_bass_kernel_spmd

    x = np.asarray(inputs["x"], dtype=np.float32)           # [1024,1,N]
    bands = _build_constants(
        inputs["deriv_w"], inputs["filt_w"], inputs["W1"], inputs["b1"],
        inputs["W2"], inputs["b2"], inputs["W3"], inputs["b3"])

    x2 = x[:, 0, :]
    xp = np.concatenate([x2[:, -PAD_L:], x2, x2[:, :NEED_R]], axis=1)
    xp = np.ascontiguousarray(xp, dtype=np.float32)

    nc = _build_bass()
    in_maps = []
    for c in range(NCORES):
        in_maps.append({
            "xp": np.ascontiguousarray(xp[c * BL:(c + 1) * BL]),
            "bands": bands,
        })
    res = run_bass_kernel_spmd(nc, in_maps, core_ids=list(range(NCORES)))
    global LAST
    LAST = res
    out = np.empty((B, 1, N), dtype=np.float32)
    for c in range(NCORES):
        out[c * BL:(c + 1) * BL, 0, :] = res.results[c]["out"]
    return out



# revision 18
# speedup vs baseline: 2.6112x; 2.6112x over previous
"""Trainium2 Bass kernel for nn_EnhancedSmagorinsky (B=1024, N=16384, f32).

Strategy (8 cores, pure batch-parallel, 128 batch rows per core):
- All convs become band-matrix matmuls in a position-major layout.
- Per tile of 107 output positions: PE-transpose a 127-position x window
  (plus a constant ones row used to inject biases through the matmuls),
  then chain: fused conv1 (deriv/box composed into one 9-tap kernel),
  conv2, conv3, the xdiff deriv, and the final deriv conv (computed as
  y_tile^T @ band, which lands row-major for direct store).
- ELU(z) computed exactly as max(z+1, min(exp(z), 1)) - 1 with the +1
  shift absorbed into the next layer's bias: 1 ScalarE exp + 1 fused
  DVE scalar_tensor_tensor per element.
- clip(elu(z),0,1) == clip(z,0,1) exactly, so layer 3 needs no exp.
"""
import json
import math

import numpy as np

# ---- problem constants (hardcoded; kernel.py must be self-contained) ----
B = 1024
N = 16384
NCORES = 8
BL = B // NCORES          # 128 batch rows per core
SCALE = 1.0
DX = SCALE / N
CUTOFF = 2.0 * DX
SQRT2 = float(np.sqrt(2.0))

# tile geometry
W = 127                   # x-window rows per tile (row 127 = ones)
S = 107                   # output positions per tile
PAD_L = 10
NT = math.ceil(N / S)     # 154
NEED_R = S * (NT - 1) + W - N
NPAD = PAD_L + N + NEED_R
GT = 4                    # tiles per group (free dim = GT*128 = 512)
NGROUPS = math.ceil(NT / GT)

# band-matrix column offsets inside the packed "bands" tensor
O_B1 = [co * 119 for co in range(6)]
O_XD = 6 * 119
O_B2 = [[O_XD + 111 + (ci * 3 + co) * 115 for co in range(3)] for ci in range(6)]
O_B3 = [O_XD + 111 + 18 * 115 + ci * 111 for ci in range(3)]
O_B5 = O_B3[0] + 3 * 111
O_ID = O_B5 + 108
O_ONES = O_ID + 128
NB = O_ONES + 512
NB16 = 216               # bf16 tensor: B5hi (108 cols) | B5lo (108 cols)


def _build_constants(deriv_w, filt_w, W1, b1, W2, b2, W3, b3):
    """Pack all band matrices into one [128, NB] f32 tensor."""
    dw = np.asarray(deriv_w).reshape(5).astype(np.float64)
    bw = np.asarray(filt_w).reshape(3).astype(np.float64)
    W1 = np.asarray(W1).astype(np.float64)
    W2 = np.asarray(W2).astype(np.float64)
    W3 = np.asarray(W3).astype(np.float64)
    b1 = np.asarray(b1).astype(np.float64)
    b2 = np.asarray(b2).astype(np.float64)
    b3 = np.asarray(b3).astype(np.float64)

    F = -bw.copy()
    F[1] += 1.0
    K1 = np.zeros((6, 9))
    for co in range(6):
        K1[co] += np.convolve(W1[co, 0], dw)
        K1[co, 1:8] += np.convolve(W1[co, 1], F)
    shift1 = b1 + 1.0
    shift2 = b2 + 1.0 - W2.sum(axis=(1, 2))
    shift3 = b3[0] + 1.0 - W3.sum()

    bands = np.zeros((128, NB))
    for co in range(6):
        Bm = bands[:, O_B1[co]: O_B1[co] + 119]
        for r1 in range(119):
            for m in range(9):
                Bm[r1 + m, r1] = K1[co, m]
        Bm[127, :] = shift1[co]
    Bm = bands[:, O_XD: O_XD + 111]
    for r in range(111):
        for m in range(5):
            Bm[r + 6 + m, r] = dw[m]
    for ci in range(6):
        for co in range(3):
            Bm = bands[:, O_B2[ci][co]: O_B2[ci][co] + 115]
            for r2 in range(115):
                for m in range(5):
                    Bm[r2 + m, r2] = W2[co, ci, m]
            if ci == 5:
                Bm[119, :] = shift2[co]
    for ci in range(3):
        Bm = bands[:, O_B3[ci]: O_B3[ci] + 111]
        for r3 in range(111):
            for m in range(5):
                Bm[r3 + m, r3] = W3[0, ci, m]
        if ci == 2:
            Bm[115, :] = shift3
    Bm = bands[:, O_B5: O_B5 + 107]
    for np_ in range(107):
        for m in range(5):
            Bm[np_ + m, np_] = dw[m] * (CUTOFF ** 2) * SQRT2
    bands[:, O_ID: O_ID + 128] = np.eye(128)
    bands[0, O_ONES: O_ONES + 512] = 1.0
    bands = bands.astype(np.float32)

    # bf16 hi/lo split of the final band: matmul cost is keyed on the rhs
    # (moving) dtype — bf16 runs 1 cyc/row vs 4 for f32 at F=108 — and the
    # accumulate promotes to f32, so hi+lo recovers ~f32 weight precision.
    import ml_dtypes
    b5f = np.zeros((128, 108), dtype=np.float32)
    b5f[:111, :107] = bands[:111, O_B5: O_B5 + 107]
    b5hi = b5f.astype(ml_dtypes.bfloat16)
    b5lo = (b5f - b5hi.astype(np.float32)).astype(ml_dtypes.bfloat16)
    bands16 = np.concatenate([b5hi, b5lo], axis=1)
    # bf16 identity for PE transposes: multiplying by exact 1.0/0.0 is
    # lossless and runs 1 cyc/row vs 2 for f32.
    identb = np.eye(128).astype(ml_dtypes.bfloat16)
    return bands, bands16, identb


# ---- BIR fix: this walrus build allows only one embedded sync-wait per
# instruction; hoist extras onto standalone EventSemaphore instructions ----
def _split_multiwait_bir(bir_bytes: bytes) -> bytes:
    bir = json.loads(bir_bytes)
    ctr = 0
    for fn in bir.get("functions", []):
        for blk in fn.get("blocks", []):
            out = []
            for inst in blk.get("instructions", []):
                si = inst.get("sync_info")
                if si:
                    waits = si.get("on_wait") or []
                    if len(waits) > 1:
                        for w in waits[:-1]:
                            ctr += 1
                            out.append({
                                "debug": inst.get("debug", 0),
                                "engine": inst["engine"],
                                "ins": [], "outs": [],
                                "name": f"xwait-{ctr}",
                                "opcode": "EventSemaphore",
                                "sync_info": {"on_update": [], "on_wait": [w]},
                            })
                        si["on_wait"] = [waits[-1]]
                out.append(inst)
            blk["instructions"] = out
    return json.dumps(bir).encode()


_CACHED_NC = None


def _build_bass():
    global _CACHED_NC
    if _CACHED_NC is not None:
        return _CACHED_NC
    from contextlib import ExitStack

    import concourse.bass as bass
    import concourse.tile as tile
    from concourse import mybir
    from concourse.alu_op_type import AluOpType

    F32 = mybir.dt.float32
    F32R = mybir.dt.float32r  # 1 cyc/row (vs 4 for f32) at F>=256; verifier needs
    # the full producer chain declared f32r, so the data tiles use it directly.
    AF = mybir.ActivationFunctionType

    nc = bass.Bass()
    xpD = nc.dram_tensor("xp", [BL, NPAD], F32R, kind="ExternalInput")
    bandsD = nc.dram_tensor("bands", [128, NB], F32R, kind="ExternalInput")
    outD = nc.dram_tensor("out", [BL, N], F32, kind="ExternalOutput")

    with ExitStack() as ctx:
        tc = ctx.enter_context(tile.TileContext(nc))
        const = ctx.enter_context(tc.tile_pool(name="const", bufs=1))
        sb = ctx.enter_context(tc.tile_pool(name="sb", bufs=2))
        ps = ctx.enter_context(tc.tile_pool(name="ps", bufs=1, space="PSUM"))

        x_sb = const.tile([BL, NPAD], F32R, tag="x_sb")
        nch = 8
        csz = (NPAD + nch - 1) // nch
        for ci_ in range(nch):
            c0 = ci_ * csz
            c1 = min(NPAD, c0 + csz)
            nc.sync.dma_start(out=x_sb[:, c0:c1], in_=xpD[:, c0:c1])
        bands = const.tile([128, NB], F32R, tag="bands")
        nc.sync.dma_start(out=bands, in_=bandsD[:])
        neg1 = const.tile([128, 1], F32, tag="neg1")
        nc.vector.memset(neg1, -1.0)
        identr = bands[:128, O_ID: O_ID + 128]
        onesr = bands[0:1, O_ONES: O_ONES + 512]

        Bxd = bands[:128, O_XD: O_XD + 111]


        # prewarm: constant ones rows live at fixed pool-slot addresses
        for _ in range(2):
            t_ = sb.tile([128, 512], F32R, tag="xpos")
            nc.sync.dma_start(out=t_[127:128, :], in_=onesr[:, :])
            t_ = sb.tile([120, 512], F32R, tag="h1_5")
            nc.sync.dma_start(out=t_[119:120, :], in_=onesr[:, :])
            t_ = sb.tile([116, 512], F32R, tag="h2_2")
            nc.sync.dma_start(out=t_[115:116, :], in_=onesr[:, :])

        def group(g):
            t0 = g * GT
            gt = min(GT, NT - t0)
            Fc = gt * 128
            # --- transpose x windows into position-major ---
            tp = ps.tile([127, 512], F32R, tag="tp")
            for j in range(gt):
                t = t0 + j
                nc.tensor.transpose(
                    tp[:, 128 * j: 128 * (j + 1)],
                    x_sb[:, S * t: S * t + W],
                    identr,
                )
            xt = sb.tile([128, 512], F32R, tag="xpos")
            nc.vector.tensor_copy(xt[:127, :Fc], tp[:127, :Fc])
            # --- xdiff ---
            xd = ps.tile([111, 512], F32, tag="xd")
            nc.tensor.matmul(xd[:, :Fc], lhsT=Bxd,
                             rhs=xt[:, :Fc],
                             start=True, stop=True)
            # --- conv1 (fused 9-tap) + ELU ---
            h1 = []
            for co in range(6):
                z = ps.tile([119, 512], F32, tag="z1", bufs=2)
                nc.tensor.matmul(
                    z[:, :Fc],
                    lhsT=bands[:128, O_B1[co]: O_B1[co] + 119],
                    rhs=xt[:, :Fc], start=True, stop=True)
                e = sb.tile([119, 512], F32, tag="e1", bufs=3)
                nc.scalar.activation(e[:, :Fc], z[:, :Fc], AF.Exp,
                                     bias=neg1[:119], scale=1.0)
                rows = 120 if co == 5 else 119
                h = sb.tile([rows, 512], F32R, tag=f"h1_{co}")
                nc.vector.scalar_tensor_tensor(
                    out=h[:119, :Fc], in0=e[:, :Fc], scalar=1.0,
                    in1=z[:, :Fc], op0=AluOpType.min, op1=AluOpType.max)
                h1.append(h)
            # --- conv2 + ELU ---
            h2 = []
            for co in range(3):
                z = ps.tile([115, 512], F32, tag="z2", bufs=2)
                for ci in range(6):
                    K = 120 if ci == 5 else 119
                    nc.tensor.matmul(
                        z[:, :Fc],
                        lhsT=bands[:K, O_B2[ci][co]: O_B2[ci][co] + 115],
                        rhs=h1[ci][:K, :Fc],
                        start=(ci == 0), stop=(ci == 5))
                e = sb.tile([115, 512], F32, tag="e2", bufs=3)
                nc.scalar.activation(e[:, :Fc], z[:, :Fc], AF.Exp,
                                     bias=neg1[:115], scale=1.0)
                rows = 116 if co == 2 else 115
                h = sb.tile([rows, 512], F32R, tag=f"h2_{co}")
                nc.vector.scalar_tensor_tensor(
                    out=h[:115, :Fc], in0=e[:, :Fc], scalar=1.0,
                    in1=z[:, :Fc], op0=AluOpType.min, op1=AluOpType.max)
                h2.append(h)
            # --- conv3; clip(elu(z),0,1) == clip(z,0,1), so no exp ---
            z3 = ps.tile([111, 512], F32, tag="z3")
            for ci in range(3):
                K = 116 if ci == 2 else 115
                nc.tensor.matmul(
                    z3[:, :Fc], lhsT=bands[:K, O_B3[ci]: O_B3[ci] + 111],
                    rhs=h2[ci][:K, :Fc],
                    start=(ci == 0), stop=(ci == 2))
            u = sb.tile([111, 512], F32, tag="u")       # cs + 1
            nc.vector.tensor_scalar(out=u[:, :Fc], in0=z3[:, :Fc],
                                    scalar1=1.0, scalar2=2.0,
                                    op0=AluOpType.max, op1=AluOpType.min)
            sq = sb.tile([111, 512], F32, tag="sq")     # cs^2
            nc.scalar.activation(sq[:, :Fc], u[:, :Fc], AF.Square,
                                 bias=neg1[:111], scale=1.0)
            ab = sb.tile([111, 512], F32, tag="ab")     # |xd|
            nc.scalar.activation(ab[:, :Fc], xd[:, :Fc], AF.Abs)
            v = sb.tile([111, 512], F32, tag="v")       # |xd|*xd
            nc.vector.tensor_tensor(v[:, :Fc], ab[:, :Fc], xd[:, :Fc],
                                    AluOpType.mult)
            y = sb.tile([111, 512], F32R, tag="y")
            nc.vector.tensor_tensor(y[:, :Fc], v[:, :Fc], sq[:, :Fc],
                                    AluOpType.mult)
            # --- final deriv conv, row-major out via y^T @ B5 ---
            ops = ps.tile([128, 512], F32, tag="ops")
            njs = []
            for j in range(gt):
                nj = min(S, N - S * (t0 + j))
                if nj <= 0:
                    continue
                n_mm = nj + (nj & 1)
                nc.tensor.matmul(
                    ops[:, 108 * j: 108 * j + n_mm],
                    lhsT=y[:111, 128 * j: 128 * j + 128],
                    rhs=bands[:111, O_B5: O_B5 + n_mm],
                    start=True, stop=True)
                njs.append(nj)
            cols = 108 * (len(njs) - 1) + njs[-1]
            osb = sb.tile([128, 440], F32, tag="osb")
            nc.vector.tensor_copy(osb[:, :cols], ops[:, :cols])
            if all(nj == S for nj in njs):
                src_ap = osb[:, :gt * 108].rearrange(
                    "p (g s) -> p g s", s=108)[:, :, :S]
                dst_ap = outD[:, S * t0: S * (t0 + gt)].rearrange(
                    "p (g s) -> p g s", s=S)
                nc.sync.dma_start(out=dst_ap, in_=src_ap)
            else:
                for j, nj in enumerate(njs):
                    nc.sync.dma_start(
                        out=outD[:, S * (t0 + j): S * (t0 + j) + nj],
                        in_=osb[:, 108 * j: 108 * j + nj])

        for g in range(NGROUPS):
            group(g)

    orig = nc.to_json_bytes
    nc.to_json_bytes = lambda: _split_multiwait_bir(orig())
    _CACHED_NC = nc
    return nc


def kernel(**inputs) -> np.ndarray:
    from concourse.bass_utils import run_bass_kernel_spmd

    x = np.asarray(inputs["x"], dtype=np.float32)           # [1024,1,N]
    bands, bands16, identb = _build_constants(
        inputs["deriv_w"], inputs["filt_w"], inputs["W1"], inputs["b1"],
        inputs["W2"], inputs["b2"], inputs["W3"], inputs["b3"])

    x2 = x[:, 0, :]
    xp = np.concatenate([x2[:, -PAD_L:], x2, x2[:, :NEED_R]], axis=1)
    xp = np.ascontiguousarray(xp, dtype=np.float32)

    nc = _build_bass()
    in_maps = []
    for c in range(NCORES):
        in_maps.append({
            "xp": np.ascontiguousarray(xp[c * BL:(c + 1) * BL]),
            "bands": bands,
        })
    res = run_bass_kernel_spmd(nc, in_maps, core_ids=list(range(NCORES)))
    global LAST
    LAST = res
    out = np.empty((B, 1, N), dtype=np.float32)
    for c in range(NCORES):
        out[c * BL:(c + 1) * BL, 0, :] = res.results[c]["out"]
    return out

